# revision 20
# baseline (speedup 1.0000x reference)
"""MoE feed-forward (top-2 routing, E=8 experts) on 8 trn2 NeuronCores.

Strategy: expert parallelism (1 expert per core).
  - Router is token-sharded (f32 for exact top-2 selection): core i routes
    tokens [1024*i, 1024*(i+1)); per-token metadata is AllGather'd.
  - Compaction is SHARDED BY TOKEN COLUMNS: every core computes global ranks
    for all 8 experts (SPMD-uniform), but scatters only ITS OWN 8 token
    columns' records (tok+1, gate) into a zeroed [E*CAP, 2] partial table
    (16 indirect-DMA calls instead of 64 — the per-call DGE cost is the
    routing bottleneck).  A ReduceScatter(add) over the expert axis then
    hands each core exactly its expert's [CAP, 2] slot table (empty slots
    sum to 0 -> tok'=0 decodes to the OOB padding marker).
  - Dispatch: indirect-DMA gather of x rows from a full replica of x.
  - Expert FFN in bf16 (full PE rate + FWL weight loads; fp32 PSUM
    accumulate): h = gelu(x @ w1 + b1); o = (h @ w2 + b2) * gate.
    w1 AND w2 are SBUF-resident, host-pretransposed for contiguous loads.
    Group g+1's x-row gathers are issued ahead of group g's output scatters
    so the PE never waits at group boundaries.
  - Combine: each core scatters weighted expert-output rows (bf16) into a
    zeroed [T, D] buffer by token id; one ReduceScatter(add) sums the two
    expert contributions per token and leaves each core its token shard.

Token layout on-chip: [128 partitions, 64 columns], token t = 128*c + p.
Slot layout (per expert): slot r lives at table row r = s*128 + p.
"""
import numpy as np
import ml_dtypes

import concourse.tile as tile
from concourse import bass, bacc, mybir
from concourse.bass_utils import run_bass_kernel_spmd
from concourse.masks import make_identity, make_upper_triangular

N_CORES = 8
P = 128
E = 8
K = 2
D = 1024
F = 2048
B, S = 4, 2048
T = B * S                  # 8192 tokens
TPC = T // N_CORES         # 1024 tokens per core
CAP = 2560                 # ceil(1.25 * T * K / E)
NSLOT_T = CAP // P         # 20 slot tiles
NCOL = T // P              # 64 token columns
CPC = NCOL // N_CORES      # 8 token columns scattered per core
GRP = 512                  # moving free dim per matmul group
NGRP = CAP // GRP          # 5 groups
DC = D // P                # 8 d-chunks
FC = F // P                # 16 f-chunks
PAD_TOK = 65536            # padding-slot marker (> T-1 -> OOB, DMA skipped)
BIG = 1.0e6                # rank>=CAP clamp -> lands OOB of [E*CAP] table
f32 = mybir.dt.float32
bf16 = mybir.dt.bfloat16
i32 = mybir.dt.int32


def build_kernel():
    nc = bacc.Bacc(num_devices=N_CORES)

    # ---------------- parameters ----------------
    x_bf = nc.declare_dram_parameter("x_bf", [T, D], bf16, isOutput=False)
    x_shard = nc.declare_dram_parameter("x_shard", [TPC, D], f32, isOutput=False)
    rw = nc.declare_dram_parameter("rw", [D, E], f32, isOutput=False)
    rb_b = nc.declare_dram_parameter("rb_b", [P, E], f32, isOutput=False)
    my_e = nc.declare_dram_parameter("my_e", [P, 1], f32, isOutput=False)
    myrow = nc.declare_dram_parameter("myrow", [E, 1], f32, isOutput=False)
    w1_p = nc.declare_dram_parameter("w1_p", [P, DC * FC * P], bf16, isOutput=False)
    b1_p = nc.declare_dram_parameter("b1_p", [P, FC], f32, isOutput=False)
    w2_p = nc.declare_dram_parameter("w2_p", [P, FC * DC * P], bf16, isOutput=False)
    b2_p = nc.declare_dram_parameter("b2_p", [P, DC], f32, isOutput=False)
    out_shard = nc.declare_dram_parameter("out_shard", [TPC, D], f32, isOutput=True)

    # ---------------- internal DRAM ----------------
    cnt_in = nc.dram_tensor("cnt_in", [E], f32)                     # my per-expert counts
    cnt_all = nc.dram_tensor("cnt_all", [N_CORES, E], f32, addr_space="Shared")
    ptable = nc.dram_tensor("ptable", [E * CAP, 2], f32)            # my columns' records
    smeta = nc.dram_tensor("smeta", [CAP, 2], f32)                  # my expert, reduced
    rs_in = nc.dram_tensor("rs_in", [T, D], bf16)                   # combine scatter buffer
    rs_out = nc.dram_tensor("rs_out", [TPC, D], bf16)               # my token shard, summed

    with tile.TileContext(nc) as tc:
        with tc.tile_pool(name="const", bufs=1) as cpool:
            ident = cpool.tile([P, P], f32)
            make_identity(nc, ident[:])
            tri = cpool.tile([P, P], f32)
            make_upper_triangular(nc, tri[:], val=1.0, diag=False)  # tri[p,i]=1 iff p<i
            ones_col = cpool.tile([P, 1], f32)
            nc.gpsimd.memset(ones_col[:], 1.0)
            ones_row1 = cpool.tile([1, P], f32)
            nc.gpsimd.memset(ones_row1[:], 1.0)
            rb_sb = cpool.tile([P, E], f32)
            nc.sync.dma_start(out=rb_sb[:], in_=rb_b.ap())
            mye_sb = cpool.tile([P, 1], f32)
            nc.sync.dma_start(out=mye_sb[:], in_=my_e.ap())
            myrow_sb = cpool.tile([E, 1], f32)
            nc.sync.dma_start(out=myrow_sb[:], in_=myrow.ap())
            rw_sb = cpool.tile([P, DC, E], f32)
            nc.sync.dma_start(out=rw_sb[:], in_=rw.ap().rearrange("(c p) e -> p c e", p=P))
            tokf = cpool.tile([P, NCOL], f32)
            toki = cpool.tile([P, NCOL], i32)
            nc.gpsimd.iota(toki[:], pattern=[[P, NCOL]], base=0, channel_multiplier=1)
            nc.vector.tensor_copy(tokf[:], toki[:])

            # ---------- router on my shard (x_shard loads get DMA priority) ----------
            meta_sb = cpool.tile([P, 4 * E], f32)
            mxs = cpool.tile([P, TPC // P, 8], f32)
            mis = cpool.tile([P, TPC // P, 8], mybir.dt.uint32)
            with tc.tile_pool(name="rt", bufs=2) as rt, \
                 tc.tile_pool(name="rtp", bufs=2, space="PSUM") as rtp:
                for g in range(TPC // P):
                    xs = rt.tile([P, D], f32, tag="xs", bufs=3)
                    nc.sync.dma_start(out=xs[:], in_=x_shard.ap()[g * P:(g + 1) * P, :])
                    xT = rt.tile([P, DC, P], f32, tag="xT")
                    for dci in range(DC):
                        tp = rtp.tile([P, P], f32, space="PSUM", tag="tp")
                        nc.tensor.transpose(out=tp[:], in_=xs[:, dci * P:(dci + 1) * P],
                                            identity=ident[:])
                        nc.vector.tensor_copy(xT[:, dci, :], tp[:])
                    lg = rtp.tile([P, E], f32, space="PSUM", tag="lg")
                    for dci in range(DC):
                        nc.tensor.matmul(out=lg[:], lhsT=xT[:, dci, :], rhs=rw_sb[:, dci, :],
                                         start=(dci == 0), stop=(dci == DC - 1))
                    lsb = rt.tile([P, E], f32, tag="lsb")
                    nc.vector.tensor_tensor(out=lsb[:], in0=lg[:], in1=rb_sb[:],
                                            op=mybir.AluOpType.add)
                    nc.vector.max_with_indices(mxs[:, g, :], mis[:, g, :], lsb[:])
                # fields: E1 | E2 | G1 | G2 at cols 0:8, 8:16, 16:24, 24:32
                nc.vector.tensor_copy(meta_sb[:, 0:E], mis[:, :, 0])
                nc.vector.tensor_copy(meta_sb[:, E:2 * E], mis[:, :, 1])
                diffs = rt.tile([P, E], f32, bufs=1)
                nc.vector.tensor_tensor(out=diffs[:], in0=mxs[:, :, 0],
                                        in1=mxs[:, :, 1],
                                        op=mybir.AluOpType.subtract)
                nc.scalar.activation(out=meta_sb[:, 2 * E:3 * E], in_=diffs[:],
                                     func=mybir.ActivationFunctionType.Sigmoid)
                nc.vector.tensor_scalar(out=meta_sb[:, 3 * E:4 * E],
                                        in0=meta_sb[:, 2 * E:3 * E],
                                        scalar1=-1.0, scalar2=1.0,
                                        op0=mybir.AluOpType.mult,
                                        op1=mybir.AluOpType.add)

                # zero the combine scatter buffer; the zseed copy makes the
                # 16MB of writes depend on router completion so they fill the
                # DMA-idle AllGather/rank window instead of jamming the router
                ZB = 8
                ztile = cpool.tile([P, ZB, D], bf16)
                nc.vector.memset(ztile[:], 0.0)
                zseed = cpool.tile([P, 1], f32)
                nc.vector.tensor_scalar(out=zseed[:],
                                        in0=meta_sb[:, 4 * E - 1:4 * E],
                                        scalar1=0.0, scalar2=None,
                                        op0=mybir.AluOpType.mult)
                for zi in range(T // (ZB * P)):
                    nc.scalar.dma_start(
                        out=rs_in.ap()[zi * ZB * P:(zi + 1) * ZB * P, :].rearrange(
                            "(b p) d -> p b d", p=P),
                        in_=ztile[:])

            # ---------- heavy loads AFTER the router (WAW dep via zseed write
            # keeps their 8.4MB out of the router's DMA window) ----------
            w1_sb = cpool.tile([P, DC, FC, P], bf16)
            nc.vector.tensor_copy(w1_sb[:, 0, 0, 0:1], zseed[:])
            nc.sync.dma_start(out=w1_sb[:], in_=w1_p.ap().rearrange(
                "p (dc fc q) -> p dc fc q", dc=DC, fc=FC))
            w2_sb = cpool.tile([P, FC, DC, P], bf16)
            nc.vector.tensor_copy(w2_sb[:, 0, 0, 0:1], zseed[:])
            nc.sync.dma_start(out=w2_sb[:], in_=w2_p.ap().rearrange(
                "p (fc dc q) -> p fc dc q", fc=FC, dc=DC))
            b1_sb = cpool.tile([P, FC], f32)
            nc.sync.dma_start(out=b1_sb[:], in_=b1_p.ap())
            b2_sb = cpool.tile([P, DC], f32)
            nc.sync.dma_start(out=b2_sb[:], in_=b2_p.ap())
            # ---------- local ranks + tiny counts AllGather ----------
            # Each core handles only ITS OWN tokens: global rank under expert e
            # = (sum of earlier cores' counts for e) + local rank.  Only the
            # [8 x 8] counts matrix crosses cores.
            with tc.tile_pool(name="mt", bufs=1) as mt, \
                 tc.tile_pool(name="mtp", bufs=2, space="PSUM") as mtp:
                NL = TPC // P            # 8 local token columns
                W = E * NL               # 64
                E1b = mt.tile([P, W], f32)
                E2b = mt.tile([P, W], f32)
                eidx = mt.tile([P, W], f32)
                nc.vector.tensor_copy(
                    E1b[:].rearrange("p (e c) -> p e c", e=E),
                    meta_sb[:, 0:E].rearrange("p (e c) -> p e c", e=1)
                    .to_broadcast([P, E, NL]))
                nc.vector.tensor_copy(
                    E2b[:].rearrange("p (e c) -> p e c", e=E),
                    meta_sb[:, E:2 * E].rearrange("p (e c) -> p e c", e=1)
                    .to_broadcast([P, E, NL]))
                for e in range(E):
                    nc.vector.memset(eidx[:, e * NL:(e + 1) * NL], float(e))
                m1b = mt.tile([P, W], f32)
                m2b = mt.tile([P, W], f32)
                maskb = mt.tile([P, W], f32)
                nc.vector.tensor_tensor(out=m1b[:], in0=E1b[:], in1=eidx[:],
                                        op=mybir.AluOpType.is_equal)
                nc.vector.tensor_tensor(out=m2b[:], in0=E2b[:], in1=eidx[:],
                                        op=mybir.AluOpType.is_equal)
                nc.vector.tensor_tensor(out=maskb[:], in0=m1b[:], in1=m2b[:],
                                        op=mybir.AluOpType.add)
                # local exclusive rank per expert block
                rps = mtp.tile([P, W], f32, space="PSUM", tag="rps")
                nc.tensor.matmul(out=rps[:], lhsT=tri[:], rhs=maskb[:],
                                 start=True, stop=False)
                cps = mtp.tile([1, W], f32, space="PSUM", tag="cps")
                nc.tensor.matmul(out=cps[:], lhsT=ones_col[:], rhs=maskb[:],
                                 start=True, stop=True)
                ctot = mt.tile([1, W], f32)
                nc.vector.tensor_copy(ctot[:], cps[:])
                cinc = mt.tile([1, W], f32)
                nc.vector.tensor_tensor_scan(out=cinc[:], data0=ctot[:], data1=ctot[:],
                                             initial=0.0, op0=mybir.AluOpType.add,
                                             op1=mybir.AluOpType.bypass)
                bases = mt.tile([1, E], f32)
                nc.vector.tensor_copy(bases[:, 1:E], cinc[0:1, NL - 1:W - NL:NL])
                nc.vector.memset(bases[:, 0:1], 0.0)
                # my per-expert counts -> AllGather
                cnt1 = mt.tile([1, E], f32)
                nc.vector.tensor_copy(cnt1[:], cinc[0:1, NL - 1:W:NL])
                nc.vector.tensor_tensor(out=cnt1[:], in0=cnt1[:], in1=bases[:],
                                        op=mybir.AluOpType.subtract)
                cnt1g = mt.tile([1, E], f32)
                nc.gpsimd.tensor_copy(cnt1g[:], cnt1[:])
                nc.gpsimd.dma_start(out=cnt_in.ap(), in_=cnt1g[:])
                nc.gpsimd.collective_compute(
                    "AllGather", mybir.AluOpType.bypass,
                    replica_groups=[list(range(N_CORES))],
                    ins=[cnt_in.ap().opt()], outs=[cnt_all.ap().opt()],
                )
                cnt8 = mt.tile([N_CORES, E], f32)
                nc.scalar.dma_start(out=cnt8[:], in_=cnt_all.ap())
                # strict-prefix over cores, then select my row
                pref_ps = mtp.tile([E, E], f32, space="PSUM", tag="pref")
                nc.tensor.matmul(out=pref_ps[:], lhsT=tri[0:E, 0:E], rhs=cnt8[:],
                                 start=True, stop=True)
                pref_sb = mt.tile([E, E], f32)
                nc.vector.tensor_copy(pref_sb[:], pref_ps[:])
                mybase_ps = mtp.tile([1, E], f32, space="PSUM", tag="mybase")
                nc.tensor.matmul(out=mybase_ps[:], lhsT=myrow_sb[:], rhs=pref_sb[:],
                                 start=True, stop=True)
                mybase = mt.tile([1, E], f32)
                nc.vector.tensor_copy(mybase[:], mybase_ps[:])
                # column base = local excl cumsum - block base + my global base
                baseb = mt.tile([1, W], f32)
                nc.vector.tensor_tensor(
                    out=baseb[:].rearrange("a (e c) -> a e c", e=E),
                    in0=mybase[:].rearrange("a (e c) -> a e c", c=1)
                    .to_broadcast([1, E, NL]),
                    in1=bases[:].rearrange("a (e c) -> a e c", c=1)
                    .to_broadcast([1, E, NL]),
                    op=mybir.AluOpType.subtract)
                cexc = mt.tile([1, W], f32)
                nc.vector.tensor_tensor(out=cexc[:], in0=cinc[:], in1=ctot[:],
                                        op=mybir.AluOpType.subtract)
                nc.vector.tensor_tensor(out=cexc[:], in0=cexc[:], in1=baseb[:],
                                        op=mybir.AluOpType.add)
                nc.tensor.matmul(out=rps[:], lhsT=ones_row1[:], rhs=cexc[:],
                                 start=False, stop=True)
                rkb = mt.tile([P, W], f32)
                nc.vector.tensor_copy(rkb[:], rps[:])
                # clamp dropped OOB, add expert segment base e*CAP
                drop = mt.tile([P, W], f32)
                nc.vector.tensor_scalar(out=drop[:], in0=rkb[:], scalar1=float(CAP),
                                        scalar2=BIG, op0=mybir.AluOpType.is_ge,
                                        op1=mybir.AluOpType.mult)
                nc.vector.tensor_tensor(out=rkb[:], in0=rkb[:], in1=drop[:],
                                        op=mybir.AluOpType.add)
                nc.vector.tensor_scalar(out=drop[:], in0=eidx[:], scalar1=float(CAP),
                                        scalar2=None, op0=mybir.AluOpType.mult)
                nc.vector.tensor_tensor(out=rkb[:], in0=rkb[:], in1=drop[:],
                                        op=mybir.AluOpType.add)
                # one-hot select across expert blocks
                pA = mt.tile([P, W], f32)
                pB = mt.tile([P, W], f32)
                nc.vector.tensor_tensor(out=pA[:], in0=rkb[:], in1=m1b[:],
                                        op=mybir.AluOpType.mult)
                nc.vector.tensor_tensor(out=pB[:], in0=rkb[:], in1=m2b[:],
                                        op=mybir.AluOpType.mult)
                for src in (pA, pB):
                    for e in range(1, E):
                        nc.vector.tensor_tensor(
                            out=src[:, 0:NL], in0=src[:, 0:NL],
                            in1=src[:, e * NL:(e + 1) * NL],
                            op=mybir.AluOpType.add)
                # payload: global token id + gates
                mye1024 = mt.tile([P, 1], f32)
                nc.vector.tensor_scalar(out=mye1024[:], in0=mye_sb[:],
                                        scalar1=float(TPC), scalar2=1.0,
                                        op0=mybir.AluOpType.mult,
                                        op1=mybir.AluOpType.add)
                tokp1 = mt.tile([P, NL], f32)
                nc.vector.tensor_scalar(out=tokp1[:], in0=tokf[:, 0:NL],
                                        scalar1=mye1024[:, 0:1], scalar2=None,
                                        op0=mybir.AluOpType.add)
                zs2 = mt.tile([P, 1], f32)
                nc.vector.tensor_scalar(out=zs2[:], in0=tokp1[:, 0:1],
                                        scalar1=0.0, scalar2=None,
                                        op0=mybir.AluOpType.mult)
                nc.vector.tensor_copy(ztile[:, 0, 0:1], zs2[:])
                oA = mt.tile([P, NL], i32)
                oB = mt.tile([P, NL], i32)
                nc.vector.tensor_copy(oA[:], pA[:, 0:NL])
                nc.vector.tensor_copy(oB[:], pB[:, 0:NL])
                payA = mt.tile([P, 2 * NL], f32)
                payB = mt.tile([P, 2 * NL], f32)
                nc.vector.tensor_copy(payA[:, 0:2 * NL:2], tokp1[:])
                nc.vector.tensor_copy(payA[:, 1:2 * NL:2], meta_sb[:, 2 * E:3 * E])
                nc.vector.tensor_copy(payB[:, 0:2 * NL:2], tokp1[:])
                nc.vector.tensor_copy(payB[:, 1:2 * NL:2], meta_sb[:, 3 * E:4 * E])

                # ----- zero partial table, 16 scatters, ReduceScatter over experts -----
                zpt = mt.tile([P, (E * CAP // P) * 2], f32)
                nc.gpsimd.memset(zpt[:], 0.0)
                nc.gpsimd.dma_start(
                    out=ptable.ap().rearrange("(p a) w -> p (a w)", p=P),
                    in_=zpt[:])
                oA_g = mt.tile([P, NL], i32)
                oB_g = mt.tile([P, NL], i32)
                nc.gpsimd.tensor_copy(oA_g[:], oA[:])
                nc.gpsimd.tensor_copy(oB_g[:], oB[:])
                payA_g = mt.tile([P, 2 * NL], f32)
                payB_g = mt.tile([P, 2 * NL], f32)
                nc.gpsimd.tensor_copy(payA_g[:], payA[:])
                nc.gpsimd.tensor_copy(payB_g[:], payB[:])
                for j in range(NL):
                    for og, pg in ((oA_g, payA_g), (oB_g, payB_g)):
                        nc.gpsimd.indirect_dma_start(
                            out=ptable.ap(),
                            out_offset=bass.IndirectOffsetOnAxis(
                                ap=og[:, j:j + 1], axis=0),
                            in_=pg[:, 2 * j:2 * j + 2],
                            in_offset=None,
                            bounds_check=E * CAP - 1,
                            oob_is_err=False,
                        )
            nc.gpsimd.collective_compute(
                "ReduceScatter", mybir.AluOpType.add,
                replica_groups=[list(range(N_CORES))],
                ins=[ptable.ap().opt()], outs=[smeta.ap().opt()],
            )

            # ---------- load + decode my slot table ----------
            slot_tok = cpool.tile([P, NSLOT_T], i32)
            slot_w = cpool.tile([P, NSLOT_T], f32)
            smf = cpool.tile([P, NSLOT_T, 2], f32)
            nc.scalar.dma_start(out=smf[:], in_=smeta.ap().rearrange(
                "(s p) w -> p s w", p=P))
            nc.vector.tensor_copy(slot_w[:], smf[:, :, 1])
            # tok = tok' - 1;  empty (tok'==0) -> PAD_TOK (OOB)
            tdec = cpool.tile([P, NSLOT_T], f32)
            empt = cpool.tile([P, NSLOT_T], f32)
            nc.vector.tensor_scalar(out=empt[:], in0=smf[:, :, 0], scalar1=0.0,
                                    scalar2=float(PAD_TOK + 1),
                                    op0=mybir.AluOpType.is_equal,
                                    op1=mybir.AluOpType.mult)
            nc.vector.tensor_scalar(out=tdec[:], in0=smf[:, :, 0], scalar1=-1.0,
                                    scalar2=None, op0=mybir.AluOpType.add)
            nc.vector.tensor_tensor(out=tdec[:], in0=tdec[:], in1=empt[:],
                                    op=mybir.AluOpType.add)
            slot_tokg = cpool.tile([P, NSLOT_T], f32)
            nc.gpsimd.tensor_copy(slot_tokg[:], tdec[:])
            nc.gpsimd.tensor_copy(slot_tok[:], slot_tokg[:])


            # =========== expert FFN (bf16, fp32 accumulate) ===========
            with tc.tile_pool(name="ffn", bufs=2) as ffn, \
                 tc.tile_pool(name="ffg", bufs=2) as ffg, \
                 tc.tile_pool(name="ffp", bufs=3, space="PSUM") as ffp:
                # gather + transpose ALL dispatch rows up front: no gather/xbar
                # traffic competes with the matmul stream afterwards
                xgTs = [ffn.tile([P, DC, GRP], bf16, name=f"xgT_{g}", bufs=1)
                        for g in range(NGRP)]
                for s in range(CAP // P):
                    xg = ffg.tile([P, D], bf16, tag="xg", bufs=4)
                    nc.gpsimd.indirect_dma_start(
                        out=xg[:], out_offset=None,
                        in_=x_bf.ap(),
                        in_offset=bass.IndirectOffsetOnAxis(
                            ap=slot_tok[:, s:s + 1], axis=0),
                        bounds_check=T - 1,
                        oob_is_err=False,
                    )
                    st = s % (GRP // P)
                    # xbar: xgT[p, dc, st*128+t] = xg[t, dc*128+p]
                    nc.sync.dma_start_transpose(
                        out=xgTs[s // (GRP // P)][:, :, st * P:(st + 1) * P],
                        in_=xg[:])
                for g in range(NGRP):
                    xgT = xgTs[g]
                    # mm1 + gelu -> hT
                    hT = ffn.tile([P, FC, GRP], bf16, tag="hT")
                    for fci in range(FC):
                        hp = ffp.tile([P, GRP], f32, space="PSUM", tag="hp")
                        for dci in range(DC):
                            nc.tensor.matmul(out=hp[:],
                                             lhsT=w1_sb[:, dci, fci, :],
                                             rhs=xgT[:, dci, :],
                                             start=(dci == 0), stop=(dci == DC - 1))
                        nc.scalar.activation(out=hT[:, fci, :], in_=hp[:],
                                             func=mybir.ActivationFunctionType.Gelu,
                                             bias=b1_sb[:, fci:fci + 1], scale=1.0)
                    # mm2 (+bias) -> oT; xbar-transpose each d-chunk immediately
                    oT = ffn.tile([P, DC, GRP], bf16, tag="oT")
                    owg = ffn.tile([P, GRP // P, D], bf16, tag="owg")
                    for dci in range(DC):
                        op = ffp.tile([P, GRP], f32, space="PSUM", tag="op")
                        for fci in range(FC):
                            nc.tensor.matmul(out=op[:],
                                             lhsT=w2_sb[:, fci, dci, :],
                                             rhs=hT[:, fci, :],
                                             start=(fci == 0), stop=(fci == FC - 1))
                        nc.vector.tensor_scalar(out=oT[:, dci, :], in0=op[:],
                                                scalar1=b2_sb[:, dci:dci + 1],
                                                scalar2=None,
                                                op0=mybir.AluOpType.add)
                        # owg[p, st, dc*128+j] = oT[j, dc, st*128+p]
                        nc.sync.dma_start_transpose(
                            out=owg[:, :, dci * P:(dci + 1) * P],
                            in_=oT[:, dci, :])
                    for st in range(GRP // P):
                        s = g * (GRP // P) + st
                        ow = ffg.tile([P, D], bf16, tag="ow", bufs=3)
                        nc.vector.tensor_scalar(out=ow[:], in0=owg[:, st, :],
                                                scalar1=slot_w[:, s:s + 1],
                                                scalar2=None,
                                                op0=mybir.AluOpType.mult)
                        nc.gpsimd.indirect_dma_start(
                            out=rs_in.ap(),
                            out_offset=bass.IndirectOffsetOnAxis(
                                ap=slot_tok[:, s:s + 1], axis=0),
                            in_=ow[:],
                            in_offset=None,
                            bounds_check=T - 1,
                            oob_is_err=False,
                        )

            # =========== combine: ReduceScatter(add) over token shards ===========
            nc.gpsimd.collective_compute(
                "ReduceScatter", mybir.AluOpType.add,
                replica_groups=[list(range(N_CORES))],
                ins=[rs_in.ap().opt()], outs=[rs_out.ap().opt()],
            )
            with tc.tile_pool(name="cmb", bufs=1) as cmb:
                HB = TPC // (2 * P)
                for h in range(2):
                    cb = cmb.tile([P, HB, D], bf16, tag="cb", bufs=2)
                    nc.sync.dma_start(out=cb[:], in_=rs_out.ap()[
                        h * HB * P:(h + 1) * HB * P, :].rearrange(
                        "(b p) d -> p b d", p=P))
                    ob = cmb.tile([P, HB, D], f32, tag="ob", bufs=2)
                    nc.vector.tensor_copy(ob[:], cb[:])
                    nc.scalar.dma_start(out=out_shard.ap()[
                        h * HB * P:(h + 1) * HB * P, :].rearrange(
                        "(b p) d -> p b d", p=P), in_=ob[:])

    nc.finalize()
    return nc


_NC_CACHE = None
TRACE = False
LAST_EXEC_NS = None
LAST_TRACE_DIR = None


def kernel(x, router_w, router_b, w1, b1, w2, b2):
    global _NC_CACHE
    x = np.ascontiguousarray(np.asarray(x, np.float32))
    router_w = np.ascontiguousarray(np.asarray(router_w, np.float32))
    router_b = np.asarray(router_b, np.float32)
    w1 = np.asarray(w1, np.float32)
    b1 = np.asarray(b1, np.float32)
    w2 = np.asarray(w2, np.float32)
    b2 = np.asarray(b2, np.float32)

    xf = x.reshape(T, D)
    xbf = np.ascontiguousarray(xf.astype(ml_dtypes.bfloat16))
    rb_b = np.tile(router_b[None, :], (P, 1))

    in_maps = []
    for c in range(N_CORES):
        w1t = np.ascontiguousarray(
            w1[c].reshape(DC, P, FC, P).transpose(1, 0, 2, 3)
        ).astype(ml_dtypes.bfloat16).reshape(P, DC * FC * P)
        w2t = np.ascontiguousarray(
            w2[c].reshape(FC, P, DC, P).transpose(1, 0, 2, 3)
        ).astype(ml_dtypes.bfloat16).reshape(P, FC * DC * P)
        myr = np.zeros((E, 1), np.float32)
        myr[c, 0] = 1.0
        in_maps.append({
            "x_bf": xbf,
            "x_shard": np.ascontiguousarray(xf[c * TPC:(c + 1) * TPC]),
            "rw": router_w,
            "rb_b": rb_b,
            "my_e": np.full((P, 1), float(c), np.float32),
            "myrow": myr,
            "w1_p": w1t,
            "b1_p": np.ascontiguousarray(b1[c].reshape(FC, P).T),
            "w2_p": w2t,
            "b2_p": np.ascontiguousarray(b2[c].reshape(DC, P).T),
        })

    global LAST_EXEC_NS, LAST_TRACE_DIR
    if _NC_CACHE is None:
        _NC_CACHE = build_kernel()
    import tempfile
    td = tempfile.mkdtemp(prefix="moe_trace_") if TRACE else None
    res = run_bass_kernel_spmd(_NC_CACHE, in_maps, list(range(N_CORES)),
                               trace=TRACE, tmpdir=td)
    LAST_EXEC_NS = getattr(res, "exec_time_ns", None)
    LAST_TRACE_DIR = td
    out = np.concatenate([res.results[c]["out_shard"] for c in range(N_CORES)], axis=0)
    return out.reshape(B, S, D)
